# revision 6
# baseline (speedup 1.0000x reference)
"""Bahdanau attention TRN2 kernel.

reference:
    proj_v = values @ W1_w + W1_b              # [B, T, U]
    proj_q = query @ W2_w + W2_b               # [B, 1, U]
    score  = tanh(proj_v + proj_q) @ V_w + V_b # [B, T, 1]
    attn   = softmax(score, axis=1)            # [B, T, 1]
    ctx    = sum(attn * values, axis=1)        # [B, D]
    returns (ctx, attn)

Sharding: data-parallel over batch, 4 batches per core on 8 cores; weights
replicated. values are uploaded pre-transposed per core as vT [4, D, T]
(layout prep on host during sharding) because both the proj matmul (contracts
D -> D on partitions) and the fused context reduction consume the [D, T]
layout; no second copy of values is needed on-chip.

Device pipeline per batch b (everything fp32):
  - proj.T tile [U128, T512] = sum_k W1_blk(k,m).T @ vT_blk(k,c)   (PE)
  - tanh(+bias) on ACT; bias[u] = W1_b + W2_b + query[b] @ W2_w (per-partition)
  - score chunk c: psum row 32c += V_j . tanh(j, c)   (PE, M=1, col-tiled,
    4 col groups run concurrently)
  - softmax over T: exp with fused accum (max-subtract skipped: |score| is
    O(1) here and softmax is shift-invariant; unused psum rows are memset to
    -1e30 so they contribute exp() = 0), Z broadcast via ones-matmul,
    reciprocal on DVE, scale on ACT.
  - context: attn chunk broadcast to 128 partitions via ones-matmul (PE),
    then fused multiply+reduce (tensor_tensor_reduce) on DVE against vT,
    accumulating over chunks into ctx [128, 4].
V_b is omitted: softmax(x + c) == softmax(x).
"""

import numpy as np
from contextlib import ExitStack

import concourse.bass as bass
import concourse.tile as tile
from concourse import bacc, mybir
from concourse.bass_utils import run_bass_kernel_spmd

B, T, D, U = 32, 2048, 512, 512
NCORES = 8
BPC = B // NCORES  # batches per core
KT = D // 128      # contraction (D) tiles
MT = U // 128      # U tiles
CT = T // 512      # T chunks
TC = 512           # T chunk size
F32 = mybir.dt.float32
AF = mybir.ActivationFunctionType
ALU = mybir.AluOpType


def _build_nc():
    nc = bacc.Bacc("TRN2", target_bir_lowering=False, debug=False)

    vT_h = nc.dram_tensor("vT", [BPC, D, T], F32, kind="ExternalInput")
    qT_h = nc.dram_tensor("qT", [D, BPC], F32, kind="ExternalInput")
    W1_h = nc.dram_tensor("W1", [D, U], F32, kind="ExternalInput")
    W2_h = nc.dram_tensor("W2", [D, U], F32, kind="ExternalInput")
    Vw_h = nc.dram_tensor("Vw", [U, 1], F32, kind="ExternalInput")
    W1b_h = nc.dram_tensor("W1b", [U], F32, kind="ExternalInput")
    W2b_h = nc.dram_tensor("W2b", [U], F32, kind="ExternalInput")
    ctx_h = nc.dram_tensor("ctx", [BPC, D], F32, kind="ExternalOutput")
    attn_h = nc.dram_tensor("attn", [BPC, T], F32, kind="ExternalOutput")

    with tile.TileContext(nc) as tc, ExitStack() as ctx:
        wpool = ctx.enter_context(tc.tile_pool(name="weights", bufs=1))
        # ---- weights / small setup ----
        w1_sb = wpool.tile([128, KT, U], F32)
        nc.sync.dma_start(out=w1_sb, in_=W1_h[:, :].rearrange("(k p) u -> p k u", p=128))
        w2_sb = wpool.tile([128, KT, U], F32)
        nc.sync.dma_start(out=w2_sb, in_=W2_h[:, :].rearrange("(k p) u -> p k u", p=128))
        v_sb = wpool.tile([128, MT], F32)
        nc.sync.dma_start(out=v_sb, in_=Vw_h[:, 0].rearrange("(j p) -> p j", p=128))
        w1b_sb = wpool.tile([128, MT], F32)
        nc.sync.dma_start(out=w1b_sb, in_=W1b_h[:].rearrange("(m p) -> p m", p=128))
        w2b_sb = wpool.tile([128, MT], F32)
        nc.sync.dma_start(out=w2b_sb, in_=W2b_h[:].rearrange("(m p) -> p m", p=128))
        qt_sb = wpool.tile([128, KT, BPC], F32)
        nc.sync.dma_start(out=qt_sb, in_=qT_h[:, :].rearrange("(k p) b -> p k b", p=128))

        ones_sb = wpool.tile([128, 128], F32)
        nc.vector.memset(ones_sb, 1.0)
        bias_sb = wpool.tile([128, MT], F32)
        nc.vector.tensor_add(bias_sb, w1b_sb, w2b_sb)

        # qb[u, m, b] = (query @ W2)[b, u] + W1_b[u] + W2_b[u]
        qb_sb = wpool.tile([128, MT, BPC], F32)
        with tc.tile_pool(name="qb_ps", bufs=1, space="PSUM") as qbpool:
            for m in range(MT):
                qb_ps = qbpool.tile([128, BPC], F32, tag="qb")
                for k in range(KT):
                    nc.tensor.matmul(
                        qb_ps,
                        lhsT=w2_sb[:, k, 128 * m : 128 * (m + 1)],
                        rhs=qt_sb[:, k, :],
                        start=(k == 0),
                        stop=(k == KT - 1),
                    )
                nc.scalar.activation(
                    out=qb_sb[:, m, :], in_=qb_ps, func=AF.Identity,
                    bias=bias_sb[:, m : m + 1], scale=1.0,
                )

        vpool = ctx.enter_context(tc.tile_pool(name="vt", bufs=2))
        tpool = ctx.enter_context(tc.tile_pool(name="tanh", bufs=6))
        spool = ctx.enter_context(tc.tile_pool(name="soft", bufs=2))
        trpool = ctx.enter_context(tc.tile_pool(name="trash", bufs=2))
        proj_pool = ctx.enter_context(tc.tile_pool(name="proj_ps", bufs=4, space="PSUM"))
        score_pool = ctx.enter_context(tc.tile_pool(name="score_ps", bufs=2, space="PSUM"))
        bcast_pool = ctx.enter_context(tc.tile_pool(name="bcast_ps", bufs=2, space="PSUM"))

        for b in range(BPC):
            vt = vpool.tile([128, KT, T], F32, tag="vt")
            for k in range(KT):
                nc.sync.dma_start(out=vt[:, k, :], in_=vT_h[b, 128 * k : 128 * (k + 1), :])

            score_ps = score_pool.tile([128, TC], F32, tag="sc")
            nc.vector.memset(score_ps, -1e30)

            for m in range(MT):
                for c in range(CT):
                    pp = proj_pool.tile([128, TC], F32, tag="pp")
                    for k in range(KT):
                        nc.tensor.matmul(
                            pp,
                            lhsT=w1_sb[:, k, 128 * m : 128 * (m + 1)],
                            rhs=vt[:, k, TC * c : TC * (c + 1)],
                            start=(k == 0),
                            stop=(k == KT - 1),
                        )
                    th = tpool.tile([128, TC], F32, tag="th")
                    nc.scalar.activation(
                        out=th, in_=pp, func=AF.Tanh,
                        bias=qb_sb[:, m, b : b + 1], scale=1.0,
                    )
                    nc.tensor.matmul(
                        score_ps[32 * c : 32 * c + 1, :],
                        lhsT=v_sb[:, m : m + 1],
                        rhs=th,
                        start=(m == 0),
                        stop=(m == MT - 1),
                        tile_position=(0, 32 * c),
                    )

            # softmax over T (rows {0,32,64,96} of score_ps; rest exp to 0)
            exp_sb = spool.tile([128, TC], F32, tag="exp")
            partials = spool.tile([128, 1], F32, tag="part")
            nc.scalar.activation(out=exp_sb, in_=score_ps, func=AF.Exp, accum_out=partials)
            z_ps = score_pool.tile([128, 1], F32, tag="sc")
            nc.tensor.matmul(z_ps, lhsT=ones_sb, rhs=partials, start=True, stop=True)
            invz = spool.tile([128, 1], F32, tag="invz")
            nc.vector.reciprocal(invz, z_ps)
            attn_sb = spool.tile([128, TC], F32, tag="attn")
            nc.scalar.mul(attn_sb, exp_sb, invz)
            nc.sync.dma_start(
                out=attn_h[b].rearrange("(c t) -> c t", t=TC),
                in_=attn_sb[0:128:32, :],
            )

            # context: ctx[p, m] = sum_{c,t} vt[p, m, c*TC+t] * attn[c, t]
            ctx_sb = spool.tile([128, MT], F32, tag="ctx")
            red = [
                spool.tile([128, CT], F32, tag=f"red{m}", name=f"red{m}_{b}")
                for m in range(MT)
            ]
            for c in range(CT):
                bc_ps = bcast_pool.tile([128, TC], F32, tag="bc")
                nc.tensor.matmul(
                    bc_ps,
                    lhsT=ones_sb[32 * c : 32 * c + 1, 0:128],
                    rhs=attn_sb[32 * c : 32 * c + 1, :],
                    start=True,
                    stop=True,
                    tile_position=(32 * c, 0),
                )
                for m in range(MT):
                    tr = trpool.tile([128, TC], F32, tag="tr")
                    nc.vector.tensor_mul(tr, vt[:, m, TC * c : TC * (c + 1)], bc_ps)
                    nc.vector.tensor_reduce(
                        out=red[m][:, c : c + 1], in_=tr,
                        axis=mybir.AxisListType.X, op=ALU.add,
                    )
            for m in range(MT):
                nc.vector.tensor_reduce(
                    out=ctx_sb[:, m : m + 1], in_=red[m],
                    axis=mybir.AxisListType.X, op=ALU.add,
                )
            nc.sync.dma_start(
                out=ctx_h[b].rearrange("(m p) -> p m", p=128),
                in_=ctx_sb,
            )

    nc.compile()
    return nc


_NC_CACHE = None


def _get_nc():
    global _NC_CACHE
    if _NC_CACHE is None:
        _NC_CACHE = _build_nc()
    return _NC_CACHE


def make_in_maps(values, query, W1_w, W1_b, W2_w, W2_b, V_w, V_b):
    values = np.ascontiguousarray(np.asarray(values, dtype=np.float32))
    query = np.asarray(query, dtype=np.float32)
    W1_w = np.ascontiguousarray(np.asarray(W1_w, dtype=np.float32))
    W2_w = np.ascontiguousarray(np.asarray(W2_w, dtype=np.float32))
    V_w = np.ascontiguousarray(np.asarray(V_w, dtype=np.float32))
    W1_b = np.ascontiguousarray(np.asarray(W1_b, dtype=np.float32))
    W2_b = np.ascontiguousarray(np.asarray(W2_b, dtype=np.float32))
    in_maps = []
    for c in range(NCORES):
        sl = slice(BPC * c, BPC * (c + 1))
        in_maps.append(
            {
                "vT": np.ascontiguousarray(values[sl].transpose(0, 2, 1)),
                "qT": np.ascontiguousarray(query[sl].T),
                "W1": W1_w,
                "W2": W2_w,
                "Vw": V_w,
                "W1b": W1_b,
                "W2b": W2_b,
            }
        )
    return in_maps


def kernel(values, query, W1_w, W1_b, W2_w, W2_b, V_w, V_b):
    nc = _get_nc()
    in_maps = make_in_maps(values, query, W1_w, W1_b, W2_w, W2_b, V_w, V_b)
    res = run_bass_kernel_spmd(nc, in_maps, core_ids=list(range(NCORES)))
    ctx = np.concatenate([r["ctx"] for r in res.results], axis=0)
    attn = np.concatenate([r["attn"] for r in res.results], axis=0)
    return ctx, attn[:, :, None]
